# revision 55
# baseline (speedup 1.0000x reference)
"""Trainium2 Bass kernel for nn_CrossModalAttention (B=4, S=2048, H=2048, single head).

Sharding: 8 cores = 4 batches x 2 sequence-halves, fully balanced (no duplicated
projections). Core c handles batch b=c//2: query-half h=c%2 for q-proj/attention/
o-proj AND key-half h for k/v projections. The pair exchanges its kTp / v' shards
via pairwise AllGather (replica_groups [[0,1],[2,3],..]) which runs on the cc
cores (overlaps compute). AG output layout = [even_rank_shard, odd_rank_shard] =
keys in natural order; softmax is key-permutation invariant anyway.

Per-core PE work: 6 x 8.6 GFLOP = 51.5 GFLOP (vs 68.7 duplicated baseline).

Single-core dataflow, all bf16 staging:
  A  kTp' = WkT.T @ krot + bk  -> k_bounce (HBM) -> AG_k -> kTp_all [2H, 1024]
     [k-rope pipelined inline on DVE at phase start]
  B  v'   = value @ WvT        -> v_bounce (HBM) -> AG_v -> v_all [2048, H]
     [q-rope on DVE overlaps]
  C  qT'  = WqT.T @ qrot + bq  -> qTp_sb (SBUF)
  D  expT = exp(scale * kTp_all-strips @ qTp); den via PE ones-matmul
  E  ctxT = (v_all-strips @ expT) * (1/den) + bv
  F  outT = WoT.T @ ctxT + bo -> HBM

cos/sin tables: query-half h and key-half h cover the same positions -> one
table pair serves both ropes, kept in SBUF across A..B.
"""

import sys

for _p in ("/opt/trn_rl_repo",):
    if _p not in sys.path:
        sys.path.append(_p)

import numpy as np

B, S, H = 4, 2048, 2048
P = 128
HO = H // P            # 16 h-tiles
SQ = S // 2            # 1024 query cols per core
SKL = S // 2           # 1024 local key rows per core
SK = S                 # 2048 keys in attention (post-gather)
NC_ = 8
SCALE = 1.0 / float(np.sqrt(H))
RG_PAIRS = [[0, 1], [2, 3], [4, 5], [6, 7]]

_PROG = None
_TRACE = False
LAST_RES = None


def _emit(nc, tile, mybir):
    F32 = mybir.dt.float32
    F16 = mybir.dt.float16
    F32R = mybir.dt.float32r
    BF16 = mybir.dt.bfloat16
    Exp = mybir.ActivationFunctionType.Exp
    Ident = mybir.ActivationFunctionType.Identity
    Bypass = mybir.AluOpType.bypass

    dram = {}
    def din(name, shape, dt=F32):
        dram[name] = nc.dram_tensor(name, list(shape), dt, kind="ExternalInput").ap()
    din("qT", (H, SQ), BF16)
    din("kT", (H, SKL), BF16)
    din("vT", (H, SKL), BF16)
    din("cos_h", (H // 2, SQ), F16); din("sin_h", (H // 2, SQ), F16)
    din("wq", (H, H), BF16); din("wk", (H, H), BF16)
    din("wv", (H, H), BF16); din("wo", (H, H), BF16)
    # biases pre-transposed host-side to [P, HO] so the DMA is one contiguous
    # 2D copy (the (t p)->p t rearrange of a flat [H] costs ~7us of 4-byte
    # descriptors on the Sync engine)
    din("bq", (P, HO)); din("bk", (P, HO)); din("bv", (P, HO)); din("bo", (P, HO))
    din("ones_col", (P, 1), BF16); din("ones_row", (1, P), F32R)
    outT = nc.dram_tensor("outT", [H, SQ], F32, kind="ExternalOutput").ap()

    def strip_ap(src2d, col0, width):
        return src2d[:, col0:col0 + width].rearrange("(o p) s -> p o s", p=P)

    def ld_strip(pool, src2d, col0, width, tag, eng=None):
        rows = src2d.shape[0]
        t = pool.tile([P, rows // P, width], src2d.dtype, name=tag)
        (eng or nc.sync).dma_start(out=t[:], in_=strip_ap(src2d, col0, width))
        return t

    with tile.TileContext(nc) as tc:
        from contextlib import ExitStack
        with ExitStack() as ctx:
            glob = ctx.enter_context(tc.tile_pool(name="glob", bufs=1))
            # one DRAM pool per tensor: DRAM-pool dependency tracking is
            # coarse, so sharing a pool makes every kTp_all reader wait for
            # AG_v (the pool's last writer) instead of just AG_k
            d_kb = ctx.enter_context(tc.tile_pool(name="d_kb", bufs=1, space="DRAM"))
            d_ka = ctx.enter_context(tc.tile_pool(name="d_ka", bufs=1, space="DRAM"))
            d_vb = ctx.enter_context(tc.tile_pool(name="d_vb", bufs=1, space="DRAM"))
            d_va = ctx.enter_context(tc.tile_pool(name="d_va", bufs=1, space="DRAM"))
            psum = ctx.enter_context(tc.tile_pool(name="psum", bufs=6, space="PSUM"))

            k_bounce = d_kb.tile([H, SKL], BF16, name="k_bounce")
            kTp_all = d_ka.tile([2 * H, SKL], BF16, name="kTp_all")
            v_bounce = d_vb.tile([SKL, H], BF16, name="v_bounce")
            v_all = d_va.tile([SK, H], BF16, name="v_all")

            bias_sb = {}
            for bn in ("bq", "bk", "bv", "bo"):
                bias_sb[bn] = glob.tile([P, HO], F32, name=f"{bn}_sb")
            ones_col = glob.tile([P, 1], BF16, name="ones_col")
            ones_row = glob.tile([1, P], F32R, name="ones_row")
            recip_bc = glob.tile([P, SQ], F32, name="recip_bc")

            def load_globals():
                for bn in ("bq", "bk", "bv", "bo"):
                    nc.sync.dma_start(out=bias_sb[bn][:], in_=dram[bn])
                nc.sync.dma_start(out=ones_col[:], in_=dram["ones_col"])
                nc.sync.dma_start(out=ones_row[:], in_=dram["ones_row"])

            HH = HO // 2
            lo, hi = slice(0, HH), slice(HH, HO)

            def rope_chunk(dst, xsrc, cs_sb, sn_sb, pool, tag, c0, cw):
                # dst[:, :, c0:c0+cw] = rope(x) using SBUF-resident cos/sin
                x = pool.tile([P, HO, cw], BF16, name=f"{tag}_in")
                nc.sync.dma_start(out=x[:], in_=strip_ap(xsrc, c0, cw))
                cs = cs_sb[:, :, c0:c0 + cw]
                sn = sn_sb[:, :, c0:c0 + cw]
                tmp = pool.tile([P, HH, cw], BF16, name=f"{tag}_tmp")
                d = dst[:, :, c0:c0 + cw]
                nc.vector.tensor_mul(d[:, lo, :], x[:, lo, :], cs)
                nc.vector.tensor_mul(tmp[:], x[:, hi, :], sn)
                nc.vector.tensor_sub(d[:, lo, :], d[:, lo, :], tmp[:])
                nc.vector.tensor_mul(d[:, hi, :], x[:, hi, :], cs)
                nc.vector.tensor_mul(tmp[:], x[:, lo, :], sn)
                nc.vector.tensor_add(d[:, hi, :], d[:, hi, :], tmp[:])

            # vT pool also hosts ALL FOUR Wv quarters: phase B runs entirely
            # under AG_k's HBM contention, so every load it needs must land
            # before the AG starts
            vT_cm = tc.tile_pool(name="vT", bufs=1)
            vTp = vT_cm.__enter__()
            vT_sb = vTp.tile([P, HO, SKL], BF16, name="vT_sb")
            wv_pre = [vTp.tile([P, HO, 512], BF16, name="wv_pre")
                      for _ in range(4)]

            # cos/sin tables in SBUF for the whole A..B window (k- and q-rope
            # use the same sequence positions on this core)
            cssn_cm = tc.tile_pool(name="cssn", bufs=1)
            cssnp = cssn_cm.__enter__()
            cs_sb = cssnp.tile([P, HH, SQ], F16, name="cs_sb")
            sn_sb = cssnp.tile([P, HH, SQ], F16, name="sn_sb")

            krot_cm = tc.tile_pool(name="krot", bufs=1)
            krotp = krot_cm.__enter__()
            krot_sb = krotp.tile([P, HO, SKL], BF16, name="krot_sb")   # 32KB

            # ---- Phase A: k-rope (inline) + k-proj (+bk) -> k_bounce ----
            with tc.tile_pool(name="wk", bufs=4) as wkp, \
                 tc.tile_pool(name="kev", bufs=4) as kev, \
                 tc.tile_pool(name="krope", bufs=2) as krope:
                # interleave cs/sn chunk loads, the rope chunks that use them,
                # and the first wk strips so the first matmul's inputs arrive
                # after ~2.5MB of DMA, not 13MB
                wk_pre = []
                for blk in range(4):
                    c0 = blk * 256
                    nc.sync.dma_start(out=cs_sb[:, :, c0:c0 + 256],
                                      in_=strip_ap(dram["cos_h"], c0, 256))
                    nc.sync.dma_start(out=sn_sb[:, :, c0:c0 + 256],
                                      in_=strip_ap(dram["sin_h"], c0, 256))
                    rope_chunk(krot_sb, dram["kT"], cs_sb, sn_sb, krope, "kr", c0, P)
                    rope_chunk(krot_sb, dram["kT"], cs_sb, sn_sb, krope, "kr",
                               c0 + P, P)
                    if blk < 2:
                        for ob in (2 * blk, 2 * blk + 1):
                            wk_pre.append(ld_strip(wkp, dram["wk"], ob * P, P, "wk_s"))
                load_globals()
                # kc OUTER so the first 16 output blocks only wait for the
                # first half of the k-rope (wk strips reloaded per kc: +8MB
                # DMA, but PE never waits for the rope tail). vT and the Wv
                # quarters (phase B) are emitted mid-loop so they land before
                # the AGs start contending for HBM but never sit in front of
                # a critical wk strip.
                for kc in range(SKL // 512):            # 2
                    for ob in range(HO):
                        if kc == 0 and ob < 4:
                            wks = wk_pre[ob]
                        else:
                            wks = ld_strip(wkp, dram["wk"], ob * P, P, "wk_s")
                        if kc == 1 and ob in (0, 2):
                            c0 = (ob // 2) * 512
                            nc.sync.dma_start(out=vT_sb[:, :, c0:c0 + 512],
                                              in_=strip_ap(dram["vT"], c0, 512))
                        if kc == 1 and ob in (4, 6, 8, 10):
                            oc = (ob - 4) // 2
                            nc.sync.dma_start(
                                out=wv_pre[oc][:],
                                in_=strip_ap(dram["wv"], oc * 512, 512))
                        ps = psum.tile([P, 512], F32, name="ps_mm")
                        for h in range(HO):
                            nc.tensor.matmul(
                                ps[:], wks[:, h, :],
                                krot_sb[:, h, kc * 512:(kc + 1) * 512],
                                start=(h == 0), stop=(h == HO - 1))
                        ev = kev.tile([P, 512], BF16, name="k_ev")
                        nc.scalar.activation(ev[:], ps[:], Ident,
                                             bias=bias_sb["bk"][:, ob:ob + 1])
                        # store issued from the (otherwise idle) GpSimd queue:
                        # under AG contention the store-issue blocks on a full
                        # DMA queue, which must not stall ACT evictions or Sync
                        nc.gpsimd.dma_start(
                            out=k_bounce[ob * P:(ob + 1) * P, kc * 512:(kc + 1) * 512],
                            in_=ev[:])
            krot_cm.__exit__(None, None, None)

            nc.gpsimd.collective_compute(
                "AllGather", Bypass, replica_groups=RG_PAIRS,
                ins=[k_bounce.opt()], outs=[kTp_all.opt()])

            # qrot: written in B (DVE), read in C; also hosts the first Wq
            # eighths so phase C starts without an AG-contended load
            qrot_cm = tc.tile_pool(name="qrot", bufs=1, side="right")
            qrotp = qrot_cm.__enter__()
            qrot_sb = qrotp.tile([P, HO, SQ], BF16, name="qrot_sb")    # 32KB
            wq_pre = [qrotp.tile([P, HO, 256], BF16, name="wq_pre")
                      for _ in range(2)]

            # v' staged fully in SBUF; bulk stores to v_bounce run decoupled
            # on GpSimd so the PE eviction chain never waits on HBM writes
            # (which drain slowly while AG_k owns the HBM)
            vloc_cm = tc.tile_pool(name="vloc", bufs=1)
            vlocp = vloc_cm.__enter__()
            v_loc = vlocp.tile([P, SKL // P, H], BF16, name="v_loc")   # 32KB

            # ---- Phase B: v-proj -> v_loc (SBUF) + q-rope on DVE ----
            with tc.tile_pool(name="qrope", bufs=2) as qrope:
                for qg in range(2):
                    nc.sync.dma_start(out=wq_pre[qg][:],
                                      in_=strip_ap(dram["wq"], qg * 256, 256))
                for c0 in range(0, SQ, P):
                    rope_chunk(qrot_sb, dram["qT"], cs_sb, sn_sb, qrope, "qr", c0, P)
                for oc in range(4):                     # 512-col groups of Wv
                    wvq = wv_pre[oc]
                    for st in range(SKL // P):          # 8 strips
                        ps = psum.tile([P, 512], F32, name="ps_mm")
                        for h in range(HO):
                            nc.tensor.matmul(
                                ps[:], vT_sb[:, h, st * P:(st + 1) * P],
                                wvq[:, h, :],
                                start=(h == 0), stop=(h == HO - 1))
                        nc.scalar.copy(v_loc[:, st, oc * 512:(oc + 1) * 512], ps[:])
                        if oc == 3:
                            nc.gpsimd.dma_start(
                                out=v_bounce[st * P:(st + 1) * P, :],
                                in_=v_loc[:, st, :])
            vloc_cm.__exit__(None, None, None)
            cssn_cm.__exit__(None, None, None)
            vT_cm.__exit__(None, None, None)

            # strips pool: serves kTp strips in D, v strips in E, and the
            # cross-phase prefetches; entered before qTp so pool-stack nesting
            # holds (qTp pops at end D, strips at end E)
            strips_cm = tc.tile_pool(name="strips", bufs=4)
            stripsp = strips_cm.__enter__()
            # both kTp halves become fully SBUF-resident during C (right after
            # AG_k completes) so phase D issues zero DMA while AG_v owns HBM
            kTpres_cm = tc.tile_pool(name="kTpres", bufs=1)
            kTpresp = kTpres_cm.__enter__()
            kTp_sb = [kTpresp.tile([P, HO, SKL], BF16, name=f"kTp{h}_sb")
                      for h in range(2)]

            # ---- Phase C: q-proj (+bq) -> qTp_sb ----
            qTp_cm = tc.tile_pool(name="qTp", bufs=1)
            qTpp = qTp_cm.__enter__()
            qTp_sb = qTpp.tile([P, HO, SQ], BF16, name="qTp_sb")       # 32KB
            with tc.tile_pool(name="wq", bufs=2) as wqp:
                # issued from GpSimd: the AG_k wait would head-of-line-block
                # Sync's wq stream
                for h in range(2):
                    nc.gpsimd.dma_start(out=kTp_sb[h][:],
                                        in_=strip_ap(kTp_all[h * H:(h + 1) * H, :],
                                                     0, SKL))
                # AG_v deliberately emitted here (gpsimd order: after the kTp
                # residency loads) so it does not own HBM while C streams Wq;
                # E does not need v_all until ~200us later
                nc.gpsimd.collective_compute(
                    "AllGather", Bypass, replica_groups=RG_PAIRS,
                    ins=[v_bounce.opt()], outs=[v_all.opt()])
                for qg in range(8):                     # 256-col groups of Wq
                    if qg < 2:
                        wqq = wq_pre[qg]
                    else:
                        wqq = wqp.tile([P, HO, 256], BF16, name="wq_q")
                        nc.sync.dma_start(out=wqq[:],
                                          in_=strip_ap(dram["wq"], qg * 256, 256))
                    for otl in range(2):
                        ot = qg * 2 + otl
                        for qc in range(2):
                            ps = psum.tile([P, 512], F32, name="ps_mm")
                            for h in range(HO):
                                nc.tensor.matmul(
                                    ps[:], wqq[:, h, otl * P:(otl + 1) * P],
                                    qrot_sb[:, h, qc * 512:(qc + 1) * 512],
                                    start=(h == 0), stop=(h == HO - 1))
                            nc.scalar.activation(
                                qTp_sb[:, ot, qc * 512:(qc + 1) * 512],
                                ps[:], Ident, bias=bias_sb["bq"][:, ot:ot + 1])
            qrot_cm.__exit__(None, None, None)

            # ---- Phase D: scores -> exp -> den (kTp strips from AG output) ----
            expT_cm = tc.tile_pool(name="expT", bufs=1, side="right")
            expTp = expT_cm.__enter__()
            expT = expTp.tile([P, SK // P, SQ], BF16, name="expT")     # 32KB
            with tc.tile_pool(name="p4den", bufs=1, space="PSUM") as p4den, \
                 tc.tile_pool(name="p4m", bufs=2) as p4m:
                _den = p4den.tile([1, 1024], F32, name="den")
                den_ps = [_den[:, 0:512], _den[:, 512:1024]]
                # den matmuls run one kt behind the score matmuls so the PE
                # (in-order) never waits on the ACT exp eviction
                pend = None
                for kt in range(SK // P):               # 16 key strips
                    half, c0 = divmod(kt * P, SKL)
                    pss = [psum.tile([P, 512], F32, name="ps_mm") for _ in range(2)]
                    for o in range(HO):
                        for qc in range(2):
                            nc.tensor.matmul(
                                pss[qc][:], kTp_sb[half][:, o, c0:c0 + P],
                                qTp_sb[:, o, qc * 512:(qc + 1) * 512],
                                start=(o == 0), stop=(o == HO - 1))
                    if pend is not None:
                        for qc in range(2):
                            nc.tensor.matmul(den_ps[qc][:], ones_col[:], pend[qc],
                                             start=(kt == 1), stop=False)
                    for qc in range(2):
                        esl = expT[:, kt, qc * 512:(qc + 1) * 512]
                        nc.scalar.activation(esl, pss[qc][:], Exp, scale=SCALE)
                    pend = [expT[:, kt, qc * 512:(qc + 1) * 512] for qc in range(2)]
                # prefetch the first two v strips for E
                vst_pre = [ld_strip(stripsp, v_all, ot * P, P, "v_strip",
                                    eng=nc.sync)
                           for ot in range(2)]
                for qc in range(2):
                    nc.tensor.matmul(den_ps[qc][:], ones_col[:], pend[qc],
                                     start=False, stop=True)
                for qc in range(2):
                    rec = p4m.tile([1, 512], F32R, name="rec")
                    with nc.allow_low_precision("fp32r is 4-byte; feeds PE broadcast"):
                        nc.vector.reciprocal(rec[:], den_ps[qc][:])
                    bc = psum.tile([P, 512], F32, name="ps_mm")
                    nc.tensor.matmul(bc[:], ones_row[:], rec[:], start=True, stop=True)
                    nc.vector.tensor_copy(recip_bc[:, qc * 512:(qc + 1) * 512], bc[:])
            qTp_cm.__exit__(None, None, None)
            kTpres_cm.__exit__(None, None, None)

            # ---- Phase E: context (v strips from AG output) ----
            ctxT_cm = tc.tile_pool(name="ctxT", bufs=1)
            ctxTp = ctxT_cm.__enter__()
            ctxT = ctxTp.tile([P, HO, SQ], BF16, name="ctxT")          # 32KB
            if True:
                for ot in range(HO):
                    if ot < 2:
                        vstrip = vst_pre[ot]
                    else:
                        vstrip = ld_strip(stripsp, v_all, ot * P, P, "v_strip")
                    pss = [psum.tile([P, 512], F32, name="ps_mm") for _ in range(2)]
                    for kt in range(SK // P):
                        for qc in range(2):
                            nc.tensor.matmul(
                                pss[qc][:], vstrip[:, kt, :],
                                expT[:, kt, qc * 512:(qc + 1) * 512],
                                start=(kt == 0), stop=(kt == SK // P - 1))
                    for qc in range(2):
                        csl = ctxT[:, ot, qc * 512:(qc + 1) * 512]
                        nc.vector.tensor_mul(csl, pss[qc][:],
                                             recip_bc[:, qc * 512:(qc + 1) * 512])
                        nc.vector.tensor_scalar_add(csl, csl,
                                                    bias_sb["bv"][:, ot:ot + 1])
                # prefetch the first two Wo strips for F
                wo_pre = [ld_strip(stripsp, dram["wo"], mt * P, P, "wo_strip")
                          for mt in range(2)]
            expT_cm.__exit__(None, None, None)

            # ---- Phase F: output projection ----
            with tc.tile_pool(name="p6o", bufs=4) as p6o:
                for mt in range(HO):
                    if mt < 2:
                        wstrip = wo_pre[mt]
                    else:
                        wstrip = ld_strip(stripsp, dram["wo"], mt * P, P, "wo_strip")
                    pss = [psum.tile([P, 512], F32, name="ps_mm") for _ in range(2)]
                    for o in range(HO):
                        for qc in range(2):
                            nc.tensor.matmul(
                                pss[qc][:], wstrip[:, o, :],
                                ctxT[:, o, qc * 512:(qc + 1) * 512],
                                start=(o == 0), stop=(o == HO - 1))
                    for qc in range(2):
                        outt = p6o.tile([P, 512], F32, name="outt")
                        nc.scalar.activation(outt[:], pss[qc][:], Ident,
                                             bias=bias_sb["bo"][:, mt:mt + 1])
                        nc.gpsimd.dma_start(
                            out=outT[mt * P:(mt + 1) * P, qc * 512:(qc + 1) * 512],
                            in_=outt[:])
            ctxT_cm.__exit__(None, None, None)
            strips_cm.__exit__(None, None, None)
    return nc


def _build():
    global _PROG
    if _PROG is not None:
        return _PROG
    import concourse.bass as bass  # noqa: F401
    import concourse.mybir as mybir
    import concourse.tile as tile
    from concourse import bacc

    nc = bacc.Bacc("TRN2", target_bir_lowering=False, debug=False, num_devices=NC_)
    _emit(nc, tile, mybir)
    nc.compile()
    _PROG = nc
    return nc


def _rope_tables():
    inv_freq = 1.0 / (10000.0 ** (np.arange(0, H, 2, dtype=np.float32) / H))
    t = np.arange(S, dtype=np.float32)
    freqs = np.outer(t, inv_freq).astype(np.float32)      # [S, H/2]
    cosT = np.ascontiguousarray(np.cos(freqs).T.astype(np.float16))  # [H/2, S]
    sinT = np.ascontiguousarray(np.sin(freqs).T.astype(np.float16))
    return cosT, sinT


def kernel(**inputs):
    nc = _build()
    from concourse.bass_utils import run_bass_kernel_spmd
    import ml_dtypes

    BF = ml_dtypes.bfloat16
    q = np.asarray(inputs["query"], dtype=np.float32)
    k = np.asarray(inputs["key"], dtype=np.float32)
    v = np.asarray(inputs["value"], dtype=np.float32)
    cosT, sinT = _rope_tables()
    wT = {n: np.ascontiguousarray(np.asarray(inputs[n], dtype=np.float32).T.astype(BF))
          for n in ("Wq", "Wk", "Wv", "Wo")}
    bias = {n: np.ascontiguousarray(
                np.asarray(inputs[n], dtype=np.float32).reshape(HO, P).T)
            for n in ("bq", "bk", "bv", "bo")}
    ones_col = np.ones((P, 1), BF)
    ones_row = np.ones((1, P), np.float32)

    in_maps = []
    for c in range(NC_):
        b, half = divmod(c, 2)
        sl = slice(half * SQ, (half + 1) * SQ)
        in_maps.append({
            "qT": np.ascontiguousarray(q[b].T[:, sl].astype(BF)),
            "kT": np.ascontiguousarray(k[b].T[:, sl].astype(BF)),
            "vT": np.ascontiguousarray(v[b].T[:, sl].astype(BF)),
            "cos_h": np.ascontiguousarray(cosT[:, sl]),
            "sin_h": np.ascontiguousarray(sinT[:, sl]),
            "wq": wT["Wq"], "wk": wT["Wk"], "wv": wT["Wv"], "wo": wT["Wo"],
            "bq": bias["bq"], "bk": bias["bk"], "bv": bias["bv"], "bo": bias["bo"],
            "ones_col": ones_col, "ones_row": ones_row,
        })

    res = run_bass_kernel_spmd(nc, in_maps, core_ids=list(range(NC_)), trace=_TRACE)
    global LAST_RES
    LAST_RES = res
    out = np.empty((B, S, H), np.float32)
    for c in range(NC_):
        b, half = divmod(c, 2)
        out[b, half * SQ:(half + 1) * SQ, :] = res.results[c]["outT"].T
    return out


# revision 60
# speedup vs baseline: 1.0536x; 1.0536x over previous
"""Trainium2 Bass kernel for nn_CrossModalAttention (B=4, S=2048, H=2048, single head).

Sharding: 8 cores = 4 batches x 2 sequence-halves, fully balanced (no duplicated
projections). Core c handles batch b=c//2: query-half h=c%2 for q-proj/attention/
o-proj AND key-half h for k/v projections. The pair exchanges its kTp / v' shards
via pairwise AllGather (replica_groups [[0,1],[2,3],..]) which runs on the cc
cores (overlaps compute). AG output layout = [even_rank_shard, odd_rank_shard] =
keys in natural order; softmax is key-permutation invariant anyway.

Per-core PE work: 6 x 8.6 GFLOP = 51.5 GFLOP (vs 68.7 duplicated baseline).

Single-core dataflow, all bf16 staging:
  A  kTp' = WkT.T @ krot + bk  -> k_bounce (HBM) -> AG_k -> kTp_all [2H, 1024]
     [k-rope pipelined inline on DVE at phase start]
  B  v'   = value @ WvT        -> v_bounce (HBM) -> AG_v -> v_all [2048, H]
     [q-rope on DVE overlaps]
  C  qT'  = WqT.T @ qrot + bq  -> qTp_sb (SBUF)
  D  expT = exp(scale * kTp_all-strips @ qTp); den via PE ones-matmul
  E  ctxT = (v_all-strips @ expT) * (1/den) + bv
  F  outT = WoT.T @ ctxT + bo -> HBM

cos/sin tables: query-half h and key-half h cover the same positions -> one
table pair serves both ropes, kept in SBUF across A..B.
"""

import sys

for _p in ("/opt/trn_rl_repo",):
    if _p not in sys.path:
        sys.path.append(_p)

import numpy as np

B, S, H = 4, 2048, 2048
P = 128
HO = H // P            # 16 h-tiles
SQ = S // 2            # 1024 query cols per core
SKL = S // 2           # 1024 local key rows per core
SK = S                 # 2048 keys in attention (post-gather)
NC_ = 8
SCALE = 1.0 / float(np.sqrt(H))
RG_PAIRS = [[0, 1], [2, 3], [4, 5], [6, 7]]

_PROG = None
_TRACE = False
LAST_RES = None


def _emit(nc, tile, mybir):
    F32 = mybir.dt.float32
    F16 = mybir.dt.float16
    F32R = mybir.dt.float32r
    BF16 = mybir.dt.bfloat16
    Exp = mybir.ActivationFunctionType.Exp
    Ident = mybir.ActivationFunctionType.Identity
    Bypass = mybir.AluOpType.bypass

    dram = {}
    def din(name, shape, dt=F32):
        dram[name] = nc.dram_tensor(name, list(shape), dt, kind="ExternalInput").ap()
    din("qT", (H, SQ), BF16)
    din("kT", (H, SKL), BF16)
    din("vT", (H, SKL), BF16)
    din("cos_h", (H // 2, SQ), F16); din("sin_h", (H // 2, SQ), F16)
    din("wq", (H, H), BF16); din("wk", (H, H), BF16)
    din("wv", (H, H), BF16); din("wo", (H, H), BF16)
    # biases pre-transposed host-side to [P, HO] so the DMA is one contiguous
    # 2D copy (the (t p)->p t rearrange of a flat [H] costs ~7us of 4-byte
    # descriptors on the Sync engine)
    din("bq", (P, HO)); din("bk", (P, HO)); din("bv", (P, HO)); din("bo", (P, HO))
    din("ones_col", (P, 1), BF16); din("ones_row", (1, P), F32R)
    outT = nc.dram_tensor("outT", [H, SQ], F32, kind="ExternalOutput").ap()

    def strip_ap(src2d, col0, width):
        return src2d[:, col0:col0 + width].rearrange("(o p) s -> p o s", p=P)

    def ld_strip(pool, src2d, col0, width, tag, eng=None):
        rows = src2d.shape[0]
        t = pool.tile([P, rows // P, width], src2d.dtype, name=tag)
        (eng or nc.sync).dma_start(out=t[:], in_=strip_ap(src2d, col0, width))
        return t

    with tile.TileContext(nc) as tc:
        from contextlib import ExitStack
        with ExitStack() as ctx:
            glob = ctx.enter_context(tc.tile_pool(name="glob", bufs=1))
            # one DRAM pool per tensor: DRAM-pool dependency tracking is
            # coarse, so sharing a pool makes every kTp_all reader wait for
            # AG_v (the pool's last writer) instead of just AG_k
            d_kb = ctx.enter_context(tc.tile_pool(name="d_kb", bufs=1, space="DRAM"))
            d_ka = ctx.enter_context(tc.tile_pool(name="d_ka", bufs=1, space="DRAM"))
            d_vb = ctx.enter_context(tc.tile_pool(name="d_vb", bufs=1, space="DRAM"))
            d_va = ctx.enter_context(tc.tile_pool(name="d_va", bufs=1, space="DRAM"))
            psum = ctx.enter_context(tc.tile_pool(name="psum", bufs=6, space="PSUM"))

            k_bounce = d_kb.tile([H, SKL], BF16, name="k_bounce")
            kTp_all = d_ka.tile([2 * H, SKL], BF16, name="kTp_all")
            v_bounce = d_vb.tile([SKL, H], BF16, name="v_bounce")
            v_all = d_va.tile([SK, H], BF16, name="v_all")

            bias_sb = {}
            for bn in ("bq", "bk", "bv", "bo"):
                bias_sb[bn] = glob.tile([P, HO], F32, name=f"{bn}_sb")
            ones_col = glob.tile([P, 1], BF16, name="ones_col")
            ones_row = glob.tile([1, P], F32R, name="ones_row")
            recip_bc = glob.tile([P, SQ], F32, name="recip_bc")

            def load_globals():
                for bn in ("bq", "bk", "bv", "bo"):
                    nc.sync.dma_start(out=bias_sb[bn][:], in_=dram[bn])
                nc.sync.dma_start(out=ones_col[:], in_=dram["ones_col"])
                nc.sync.dma_start(out=ones_row[:], in_=dram["ones_row"])

            HH = HO // 2
            lo, hi = slice(0, HH), slice(HH, HO)

            def rope_chunk(dst, xsrc, cs_sb, sn_sb, pool, tag, c0, cw):
                # dst[:, :, c0:c0+cw] = rope(x) using SBUF-resident cos/sin
                x = pool.tile([P, HO, cw], BF16, name=f"{tag}_in")
                nc.sync.dma_start(out=x[:], in_=strip_ap(xsrc, c0, cw))
                cs = cs_sb[:, :, c0:c0 + cw]
                sn = sn_sb[:, :, c0:c0 + cw]
                tmp = pool.tile([P, HH, cw], BF16, name=f"{tag}_tmp")
                d = dst[:, :, c0:c0 + cw]
                nc.vector.tensor_mul(d[:, lo, :], x[:, lo, :], cs)
                nc.vector.tensor_mul(tmp[:], x[:, hi, :], sn)
                nc.vector.tensor_sub(d[:, lo, :], d[:, lo, :], tmp[:])
                nc.vector.tensor_mul(d[:, hi, :], x[:, hi, :], cs)
                nc.vector.tensor_mul(tmp[:], x[:, lo, :], sn)
                nc.vector.tensor_add(d[:, hi, :], d[:, hi, :], tmp[:])

            # vT pool also hosts ALL FOUR Wv quarters: phase B runs entirely
            # under AG_k's HBM contention, so every load it needs must land
            # before the AG starts
            vT_cm = tc.tile_pool(name="vT", bufs=1)
            vTp = vT_cm.__enter__()
            vT_sb = vTp.tile([P, HO, SKL], BF16, name="vT_sb")
            # oc3's quarter deliberately aliases wv_pre0's slot (same name,
            # bufs=1): its anti-dep is oc0's matmuls, done early in B, so the
            # reload still lands well before oc3 needs it -- saves 16KB
            wv_pre = [vTp.tile([P, HO, 512], BF16, name=f"wv_pre{i % 3}")
                      for i in range(4)]

            # cos/sin tables in SBUF for the whole A..B window (k- and q-rope
            # use the same sequence positions on this core)
            cssn_cm = tc.tile_pool(name="cssn", bufs=1)
            cssnp = cssn_cm.__enter__()
            cs_sb = cssnp.tile([P, HH, SQ], F16, name="cs_sb")
            sn_sb = cssnp.tile([P, HH, SQ], F16, name="sn_sb")

            krot_cm = tc.tile_pool(name="krot", bufs=1)
            krotp = krot_cm.__enter__()
            krot_sb = krotp.tile([P, HO, SKL], BF16, name="krot_sb")   # 32KB

            # ---- Phase A: k-rope (inline) + k-proj (+bk) -> k_bounce ----
            with tc.tile_pool(name="wk", bufs=4) as wkp, \
                 tc.tile_pool(name="kev", bufs=4) as kev, \
                 tc.tile_pool(name="krope", bufs=2) as krope:
                # interleave cs/sn chunk loads, the rope chunks that use them,
                # and the first wk strips so the first matmul's inputs arrive
                # after ~2.5MB of DMA, not 13MB
                wk_pre = []
                for blk in range(4):
                    c0 = blk * 256
                    nc.sync.dma_start(out=cs_sb[:, :, c0:c0 + 256],
                                      in_=strip_ap(dram["cos_h"], c0, 256))
                    nc.sync.dma_start(out=sn_sb[:, :, c0:c0 + 256],
                                      in_=strip_ap(dram["sin_h"], c0, 256))
                    rope_chunk(krot_sb, dram["kT"], cs_sb, sn_sb, krope, "kr", c0, P)
                    rope_chunk(krot_sb, dram["kT"], cs_sb, sn_sb, krope, "kr",
                               c0 + P, P)
                    if blk < 2:
                        for ob in (2 * blk, 2 * blk + 1):
                            wk_pre.append(ld_strip(wkp, dram["wk"], ob * P, P, "wk_s"))
                load_globals()
                # kc OUTER so the first 16 output blocks only wait for the
                # first half of the k-rope (wk strips reloaded per kc: +8MB
                # DMA, but PE never waits for the rope tail). vT and the Wv
                # quarters (phase B) are emitted mid-loop so they land before
                # the AGs start contending for HBM but never sit in front of
                # a critical wk strip.
                for kc in range(SKL // 512):            # 2
                    for ob in range(HO):
                        if kc == 0 and ob < 4:
                            wks = wk_pre[ob]
                        else:
                            wks = ld_strip(wkp, dram["wk"], ob * P, P, "wk_s")
                        if kc == 1 and ob in (0, 2):
                            c0 = (ob // 2) * 512
                            nc.sync.dma_start(out=vT_sb[:, :, c0:c0 + 512],
                                              in_=strip_ap(dram["vT"], c0, 512))
                        if kc == 1 and ob in (4, 6, 8):
                            oc = (ob - 4) // 2
                            nc.sync.dma_start(
                                out=wv_pre[oc][:],
                                in_=strip_ap(dram["wv"], oc * 512, 512))
                        ps = psum.tile([P, 512], F32, name="ps_mm")
                        for h in range(HO):
                            nc.tensor.matmul(
                                ps[:], wks[:, h, :],
                                krot_sb[:, h, kc * 512:(kc + 1) * 512],
                                start=(h == 0), stop=(h == HO - 1))
                        ev = kev.tile([P, 512], BF16, name="k_ev")
                        nc.scalar.activation(ev[:], ps[:], Ident,
                                             bias=bias_sb["bk"][:, ob:ob + 1])
                        # store issued from the (otherwise idle) GpSimd queue:
                        # under AG contention the store-issue blocks on a full
                        # DMA queue, which must not stall ACT evictions or Sync
                        nc.gpsimd.dma_start(
                            out=k_bounce[ob * P:(ob + 1) * P, kc * 512:(kc + 1) * 512],
                            in_=ev[:])
            krot_cm.__exit__(None, None, None)

            nc.gpsimd.collective_compute(
                "AllGather", Bypass, replica_groups=RG_PAIRS,
                ins=[k_bounce.opt()], outs=[kTp_all.opt()])

            # qrot: written in B (DVE), read in C; also hosts the first Wq
            # eighths so phase C starts without an AG-contended load
            qrot_cm = tc.tile_pool(name="qrot", bufs=1, side="right")
            qrotp = qrot_cm.__enter__()
            qrot_sb = qrotp.tile([P, HO, SQ], BF16, name="qrot_sb")    # 32KB
            wq_pre = [qrotp.tile([P, HO, 256], BF16, name=f"wq_pre{i}")
                      for i in range(2)]

            # v' staged fully in SBUF; bulk stores to v_bounce run decoupled
            # on GpSimd so the PE eviction chain never waits on HBM writes
            # (which drain slowly while AG_k owns the HBM)
            vloc_cm = tc.tile_pool(name="vloc", bufs=1)
            vlocp = vloc_cm.__enter__()
            v_loc = vlocp.tile([P, SKL // P, H], BF16, name="v_loc")   # 32KB

            # ---- Phase B: v-proj -> v_loc (SBUF) + q-rope on DVE ----
            with tc.tile_pool(name="qrope", bufs=1) as qrope:
                nc.sync.dma_start(out=wv_pre[3][:],
                                  in_=strip_ap(dram["wv"], 3 * 512, 512))
                for qg in range(2):
                    nc.sync.dma_start(out=wq_pre[qg][:],
                                      in_=strip_ap(dram["wq"], qg * 256, 256))
                for c0 in range(0, SQ, P):
                    rope_chunk(qrot_sb, dram["qT"], cs_sb, sn_sb, qrope, "qr", c0, P)
                for oc in range(4):                     # 512-col groups of Wv
                    wvq = wv_pre[oc]
                    for st in range(SKL // P):          # 8 strips
                        ps = psum.tile([P, 512], F32, name="ps_mm")
                        for h in range(HO):
                            nc.tensor.matmul(
                                ps[:], vT_sb[:, h, st * P:(st + 1) * P],
                                wvq[:, h, :],
                                start=(h == 0), stop=(h == HO - 1))
                        nc.scalar.copy(v_loc[:, st, oc * 512:(oc + 1) * 512], ps[:])
                        if oc == 3:
                            nc.gpsimd.dma_start(
                                out=v_bounce[st * P:(st + 1) * P, :],
                                in_=v_loc[:, st, :])
            vloc_cm.__exit__(None, None, None)
            cssn_cm.__exit__(None, None, None)
            vT_cm.__exit__(None, None, None)

            # strips pool: serves kTp strips in D, v strips in E, and the
            # cross-phase prefetches; entered before qTp so pool-stack nesting
            # holds (qTp pops at end D, strips at end E)
            strips_cm = tc.tile_pool(name="strips", bufs=4)
            stripsp = strips_cm.__enter__()
            # both kTp halves become fully SBUF-resident during C (right after
            # AG_k completes) so phase D issues zero DMA while AG_v owns HBM
            kTpres_cm = tc.tile_pool(name="kTpres", bufs=1)
            kTpresp = kTpres_cm.__enter__()
            kTp_sb = [kTpresp.tile([P, HO, SKL], BF16, name=f"kTp{h}_sb")
                      for h in range(2)]

            # ---- Phase C: q-proj (+bq) -> qTp_sb ----
            qTp_cm = tc.tile_pool(name="qTp", bufs=1)
            qTpp = qTp_cm.__enter__()
            qTp_sb = qTpp.tile([P, HO, SQ], BF16, name="qTp_sb")       # 32KB
            with tc.tile_pool(name="wq", bufs=2) as wqp:
                # issued from GpSimd: the AG_k wait would head-of-line-block
                # Sync's wq stream
                for h in range(2):
                    nc.gpsimd.dma_start(out=kTp_sb[h][:],
                                        in_=strip_ap(kTp_all[h * H:(h + 1) * H, :],
                                                     0, SKL))
                # AG_v deliberately emitted here (gpsimd order: after the kTp
                # residency loads) so it does not own HBM while C streams Wq;
                # E does not need v_all until ~200us later
                nc.gpsimd.collective_compute(
                    "AllGather", Bypass, replica_groups=RG_PAIRS,
                    ins=[v_bounce.opt()], outs=[v_all.opt()])
                for qg in range(8):                     # 256-col groups of Wq
                    if qg < 2:
                        wqq = wq_pre[qg]
                    else:
                        wqq = wqp.tile([P, HO, 256], BF16, name="wq_q")
                        nc.sync.dma_start(out=wqq[:],
                                          in_=strip_ap(dram["wq"], qg * 256, 256))
                    for otl in range(2):
                        ot = qg * 2 + otl
                        for qc in range(2):
                            ps = psum.tile([P, 512], F32, name="ps_mm")
                            for h in range(HO):
                                nc.tensor.matmul(
                                    ps[:], wqq[:, h, otl * P:(otl + 1) * P],
                                    qrot_sb[:, h, qc * 512:(qc + 1) * 512],
                                    start=(h == 0), stop=(h == HO - 1))
                            nc.scalar.activation(
                                qTp_sb[:, ot, qc * 512:(qc + 1) * 512],
                                ps[:], Ident, bias=bias_sb["bq"][:, ot:ot + 1])
            qrot_cm.__exit__(None, None, None)

            # ---- Phase D: scores -> exp -> den (kTp strips from AG output) ----
            expT_cm = tc.tile_pool(name="expT", bufs=1, side="right")
            expTp = expT_cm.__enter__()
            expT = expTp.tile([P, SK // P, SQ], BF16, name="expT")     # 32KB
            with tc.tile_pool(name="p4den", bufs=1, space="PSUM") as p4den, \
                 tc.tile_pool(name="p4m", bufs=2) as p4m:
                _den = p4den.tile([1, 1024], F32, name="den")
                den_ps = [_den[:, 0:512], _den[:, 512:1024]]
                # den matmuls run one kt behind the score matmuls so the PE
                # (in-order) never waits on the ACT exp eviction
                pend = None
                for kt in range(SK // P):               # 16 key strips
                    half, c0 = divmod(kt * P, SKL)
                    pss = [psum.tile([P, 512], F32, name="ps_mm") for _ in range(2)]
                    for o in range(HO):
                        for qc in range(2):
                            nc.tensor.matmul(
                                pss[qc][:], kTp_sb[half][:, o, c0:c0 + P],
                                qTp_sb[:, o, qc * 512:(qc + 1) * 512],
                                start=(o == 0), stop=(o == HO - 1))
                    if pend is not None:
                        for qc in range(2):
                            nc.tensor.matmul(den_ps[qc][:], ones_col[:], pend[qc],
                                             start=(kt == 1), stop=False)
                    for qc in range(2):
                        esl = expT[:, kt, qc * 512:(qc + 1) * 512]
                        nc.scalar.activation(esl, pss[qc][:], Exp, scale=SCALE)
                    pend = [expT[:, kt, qc * 512:(qc + 1) * 512] for qc in range(2)]
                # prefetch the first two v strips for E
                vst_pre = [ld_strip(stripsp, v_all, ot * P, P, "v_strip",
                                    eng=nc.sync)
                           for ot in range(2)]
                for qc in range(2):
                    nc.tensor.matmul(den_ps[qc][:], ones_col[:], pend[qc],
                                     start=False, stop=True)
                for qc in range(2):
                    rec = p4m.tile([1, 512], F32R, name="rec")
                    with nc.allow_low_precision("fp32r is 4-byte; feeds PE broadcast"):
                        nc.vector.reciprocal(rec[:], den_ps[qc][:])
                    bc = psum.tile([P, 512], F32, name="ps_mm")
                    nc.tensor.matmul(bc[:], ones_row[:], rec[:], start=True, stop=True)
                    nc.vector.tensor_copy(recip_bc[:, qc * 512:(qc + 1) * 512], bc[:])
            qTp_cm.__exit__(None, None, None)
            kTpres_cm.__exit__(None, None, None)

            # ---- Phase E: context (v strips from AG output) ----
            ctxT_cm = tc.tile_pool(name="ctxT", bufs=1)
            ctxTp = ctxT_cm.__enter__()
            ctxT = ctxTp.tile([P, HO, SQ], BF16, name="ctxT")          # 32KB
            if True:
                for ot in range(HO):
                    if ot < 2:
                        vstrip = vst_pre[ot]
                    else:
                        vstrip = ld_strip(stripsp, v_all, ot * P, P, "v_strip")
                    pss = [psum.tile([P, 512], F32, name="ps_mm") for _ in range(2)]
                    for kt in range(SK // P):
                        for qc in range(2):
                            nc.tensor.matmul(
                                pss[qc][:], vstrip[:, kt, :],
                                expT[:, kt, qc * 512:(qc + 1) * 512],
                                start=(kt == 0), stop=(kt == SK // P - 1))
                    for qc in range(2):
                        csl = ctxT[:, ot, qc * 512:(qc + 1) * 512]
                        nc.vector.tensor_mul(csl, pss[qc][:],
                                             recip_bc[:, qc * 512:(qc + 1) * 512])
                        nc.vector.tensor_scalar_add(csl, csl,
                                                    bias_sb["bv"][:, ot:ot + 1])
                # prefetch the first two Wo strips for F
                wo_pre = [ld_strip(stripsp, dram["wo"], mt * P, P, "wo_strip")
                          for mt in range(2)]
            expT_cm.__exit__(None, None, None)

            # ---- Phase F: output projection ----
            with tc.tile_pool(name="p6o", bufs=4) as p6o:
                for mt in range(HO):
                    if mt < 2:
                        wstrip = wo_pre[mt]
                    else:
                        wstrip = ld_strip(stripsp, dram["wo"], mt * P, P, "wo_strip")
                    pss = [psum.tile([P, 512], F32, name="ps_mm") for _ in range(2)]
                    for o in range(HO):
                        for qc in range(2):
                            nc.tensor.matmul(
                                pss[qc][:], wstrip[:, o, :],
                                ctxT[:, o, qc * 512:(qc + 1) * 512],
                                start=(o == 0), stop=(o == HO - 1))
                    for qc in range(2):
                        outt = p6o.tile([P, 512], F32, name="outt")
                        nc.scalar.activation(outt[:], pss[qc][:], Ident,
                                             bias=bias_sb["bo"][:, mt:mt + 1])
                        nc.gpsimd.dma_start(
                            out=outT[mt * P:(mt + 1) * P, qc * 512:(qc + 1) * 512],
                            in_=outt[:])
            ctxT_cm.__exit__(None, None, None)
            strips_cm.__exit__(None, None, None)
    return nc


def _build():
    global _PROG
    if _PROG is not None:
        return _PROG
    import concourse.bass as bass  # noqa: F401
    import concourse.mybir as mybir
    import concourse.tile as tile
    from concourse import bacc

    nc = bacc.Bacc("TRN2", target_bir_lowering=False, debug=False, num_devices=NC_)
    _emit(nc, tile, mybir)
    nc.compile()
    _PROG = nc
    return nc


def _rope_tables():
    inv_freq = 1.0 / (10000.0 ** (np.arange(0, H, 2, dtype=np.float32) / H))
    t = np.arange(S, dtype=np.float32)
    freqs = np.outer(t, inv_freq).astype(np.float32)      # [S, H/2]
    cosT = np.ascontiguousarray(np.cos(freqs).T.astype(np.float16))  # [H/2, S]
    sinT = np.ascontiguousarray(np.sin(freqs).T.astype(np.float16))
    return cosT, sinT


def kernel(**inputs):
    nc = _build()
    from concourse.bass_utils import run_bass_kernel_spmd
    import ml_dtypes

    BF = ml_dtypes.bfloat16
    q = np.asarray(inputs["query"], dtype=np.float32)
    k = np.asarray(inputs["key"], dtype=np.float32)
    v = np.asarray(inputs["value"], dtype=np.float32)
    cosT, sinT = _rope_tables()
    wT = {n: np.ascontiguousarray(np.asarray(inputs[n], dtype=np.float32).T.astype(BF))
          for n in ("Wq", "Wk", "Wv", "Wo")}
    bias = {n: np.ascontiguousarray(
                np.asarray(inputs[n], dtype=np.float32).reshape(HO, P).T)
            for n in ("bq", "bk", "bv", "bo")}
    ones_col = np.ones((P, 1), BF)
    ones_row = np.ones((1, P), np.float32)

    in_maps = []
    for c in range(NC_):
        b, half = divmod(c, 2)
        sl = slice(half * SQ, (half + 1) * SQ)
        in_maps.append({
            "qT": np.ascontiguousarray(q[b].T[:, sl].astype(BF)),
            "kT": np.ascontiguousarray(k[b].T[:, sl].astype(BF)),
            "vT": np.ascontiguousarray(v[b].T[:, sl].astype(BF)),
            "cos_h": np.ascontiguousarray(cosT[:, sl]),
            "sin_h": np.ascontiguousarray(sinT[:, sl]),
            "wq": wT["Wq"], "wk": wT["Wk"], "wv": wT["Wv"], "wo": wT["Wo"],
            "bq": bias["bq"], "bk": bias["bk"], "bv": bias["bv"], "bo": bias["bo"],
            "ones_col": ones_col, "ones_row": ones_row,
        })

    res = run_bass_kernel_spmd(nc, in_maps, core_ids=list(range(NC_)), trace=_TRACE)
    global LAST_RES
    LAST_RES = res
    out = np.empty((B, S, H), np.float32)
    for c in range(NC_):
        b, half = divmod(c, 2)
        out[b, half * SQ:(half + 1) * SQ, :] = res.results[c]["outT"].T
    return out
